# revision 1
# baseline (speedup 1.0000x reference)
"""DiffusionGraphConv Trainium2 kernel (8-core SPMD, data-parallel over batch).

Math refactoring (halves the big-matmul FLOPs vs the reference order):
  reference: out[b,n,o] = sum_{f,m} mats_m[n,f,b] * W[f*5+m, o]
  with mats = [x0, s0 x0, 2 s0^2 x0 - x0, s1 x0, 2 s1^2 x0 - x0].
  Projection (width F=128 -> O=64) commutes with the node-space diffusion, so:
    u_m = proj(x0, W_m)                       # [N, O, B] each, cheap
    out = (u0 - u2 - u4) + s0 (u1 + 2 s0 u2) + s1 (u3 + 2 s1 u4)
  Device computes: v0 = proj(x0, W0-W2-W4), and with pre-scaled 2*W2 / 2*W4:
    c0 = u1 + s0 @ u2s ; c1 = u3 + s1 @ u4s ; out = v0 + s0 @ c0 + s1 @ c1

Per-core work: 4 matmuls [2048,2048]@[2048,512] (bf16, fp32 PSUM) + projections.
Layouts (host-prepared, all "SBUF images"):
  x0t  [128 f, 16t*8b*128j] bf16: x0t[f, (t*8+b)*128+j] = cat(inputs,state)[b, t*128+j, f]
  wcat [128 f, 5*64]        bf16: [W0-W2-W4 | W1 | 2*W2 | W3 | 2*W4]
  s*t  [16 t, 128 p, 2048]  bf16: s*t[t, p, kt*128+j] = s[t*128+j, kt*128+p]
       (strip t = transposed rows of s for output-node tile t, k-major)
  out  [2048 n, 8b*64o]     f32

Env quirks handled here: walrus accepts <=1 sync-wait per instruction
(_legalize_waits hoists extras onto EventSemaphore carriers; simulators need
legalize=False); repeat=N re-runs the idempotent pipeline for wall-clock
differencing since this axon terminal has no NTFF profiling.
"""

import sys

if "/opt/trn_rl_repo" not in sys.path:
    sys.path.insert(0, "/opt/trn_rl_repo")

import numpy as np
import ml_dtypes

import concourse.bass as bass
import concourse.mybir as mybir
from concourse.tile import TileContext
from concourse.bass_utils import run_bass_kernel_spmd

BF16 = mybir.dt.bfloat16
FP8 = mybir.dt.float8e4
NPFP8 = ml_dtypes.float8_e4m3
SCALE = 256.0
F32 = mybir.dt.float32
NPBF16 = ml_dtypes.bfloat16

N = 2048          # graph nodes
F = 128           # input_size (64 input + 64 hidden)
B = 64            # global batch
NCORES = 8
BS = B // NCORES  # 8 batches per core
O = 64            # output features
NT = N // 128     # 16 node tiles
M5 = 5            # diffusion matrices


def _legalize_waits(nc, max_waits=1):
    """Walrus in this env encodes at most one sync-wait per instruction.

    Tile's sem assignment can emit 2-3 waits on one instruction; hoist the
    excess onto standalone EventSemaphore carriers (same engine, inserted
    just before), which the sequencer executes in order — semantics are
    identical, encoding is legal."""
    f = nc.m.functions[0]
    for blk in f.blocks:
        new_insts = []
        changed = False
        for inst in blk.instructions:
            si = inst.sync_info
            waits = list(si.on_wait) if si is not None else []
            if len(waits) > max_waits:
                for i, w in enumerate(waits[:-max_waits]):
                    ev = mybir.InstEventSemaphore(
                        name=f"{inst.name}-wsplit{i}",
                        engine=inst.engine,
                        ins=[],
                        outs=[],
                        sync_info=mybir.SyncInfo(on_wait=[w], on_update=[]),
                    )
                    new_insts.append(ev)
                inst.sync_info = mybir.SyncInfo(
                    on_wait=waits[-max_waits:], on_update=list(si.on_update)
                )
                changed = True
            new_insts.append(inst)
        if changed:
            blk.instructions = new_insts
    return nc


def build_bass(n=N, bs=BS, o=O, legalize=True, n_hops=4, repeat=1):
    """Build the per-core SPMD Bass program."""
    nt = n // 128
    nc = bass.Bass()
    x0t = nc.dram_tensor("x0t", [F, bs * n], BF16, kind="ExternalInput")
    wcat = nc.dram_tensor("wcat", [F, M5 * o], BF16, kind="ExternalInput")
    s0t = nc.dram_tensor("s0t", [nt, 128, n], FP8, kind="ExternalInput")
    s1t = nc.dram_tensor("s1t", [nt, 128, n], FP8, kind="ExternalInput")
    out = nc.dram_tensor("out", [n, bs * o], F32, kind="ExternalOutput")

    obs = bs * o        # 512: width of diffusion operands
    with TileContext(nc) as tc:
        with (
            tc.tile_pool(name="persist", bufs=1) as persist,
            tc.tile_pool(name="stream", bufs=6) as stream,
            tc.tile_pool(name="pproj", bufs=4, space="PSUM") as pproj,
            tc.tile_pool(name="pacc", bufs=4, space="PSUM") as pacc,
        ):
            w_sb = persist.tile([F, M5 * o], BF16, name="w_sb")
            nc.sync.dma_start(out=w_sb[:, :], in_=wcat[:, :])
            # x0t is t-major on host: free index = t*bs*128 + b*128 + j, so
            # each node-tile's stationary slices arrive in one chunk DMA.
            x0_sb = persist.tile([F, bs * n], BF16, name="x0_sb")
            for t in range(nt):
                nc.sync.dma_start(
                    out=x0_sb[:, t * bs * 128:(t + 1) * bs * 128],
                    in_=x0t[:, t * bs * 128:(t + 1) * bs * 128],
                )
            # U[t]: [128, bs*5*o] bf16, b-major: free = b*320 + mi*64 + oo.
            # Slots mi: 0=v0, 1=u1->c0, 2=2*u2, 3=u3->c1, 4=2*u4.
            U = [
                persist.tile([128, 4 * 2 * obs], FP8, name=f"u{tp}", tag=f"u{tp}")
                for tp in range(nt // 2)
            ]
            V0 = [
                persist.tile([128, obs], BF16, name=f"w0_{t}", tag=f"w0_{t}")
                for t in range(nt)
            ]
            # V[t]: [128, obs] f32 accumulator, created in the first V-hop.
            V = [
                persist.tile([128, obs], F32, name=f"v{t}", tag=f"v{t}")
                for t in range(nt)
            ]

            def upair(tp, mi):
                """[128, 2, obs] DoubleRow moving view: k-tile pair of slot mi."""
                return U[tp].rearrange("p (mi4 kt2 c) -> p mi4 kt2 c", mi4=4, kt2=2)[
                    :, mi - 1, :, :
                ]

            def uslot_w(t, mi):
                """[128, obs] contiguous write view of slot mi for node-tile t."""
                base = (mi - 1) * 2 * obs + (t % 2) * obs
                return U[t // 2][:, base:base + obs]

            # ---- Phase 1: projections, node-tile outer so U[t] completes
            # early and hop-1 PSUM groups can close while P1 still runs.
            #   psum[:, h*512 : h*512+320] = x0_tile(b).T @ wcat   (n on psum partitions)
            def phase1(t):
                # one-bank psum per b (bufs=4 rotation) keeps PE from stalling
                # on copy drains; copies alternate DVE/ACT to pipeline at 2x
                for b in range(bs):
                    ps = pproj.tile([128, 512], F32, name="ps_proj", tag="proj")
                    nc.tensor.matmul(
                        ps[:, 0:M5 * o],
                        lhsT=x0_sb[:, (t * bs + b) * 128:(t * bs + b + 1) * 128],
                        rhs=w_sb[:, :],
                        start=True,
                        stop=True,
                    )
                    du = U[t // 2].rearrange(
                        "p (mi4 kt2 c) -> p mi4 kt2 c", mi4=4, kt2=2
                    )[:, :, t % 2, b * o:(b + 1) * o]
                    su = ps[:, o:M5 * o].rearrange("p (mi4 oo) -> p mi4 oo", oo=o)
                    if b % 2 == 0:
                        nc.vector.tensor_copy(out=V0[t][:, b * o:(b + 1) * o], in_=ps[:, 0:o])
                        nc.vector.tensor_copy(out=du, in_=su)
                    else:
                        nc.scalar.copy(out=V0[t][:, b * o:(b + 1) * o], in_=ps[:, 0:o])
                        nc.scalar.copy(out=du, in_=su)

            # ---- Phases 2-5: diffusion hops.
            #   hop(s, src_slot, dst):  for each node-tile t:
            #     acc = sum_kt sT_strip[t,kt].T @ U[kt][src_slot]   (= (s @ u)[t-tile])
            def hop(sdram, src, dst_slot, first_v, final, split_k=1):
                for t in range(nt):
                    strip = stream.tile([128, n], FP8, name="strip", tag="strip")
                    nc.sync.dma_start(out=strip[:, :], in_=sdram[t])
                    # split_k>1: independent psum sub-groups over kt ranges, so
                    # early sub-groups can close while upstream U tiles are
                    # still being produced (fills PE idle at phase boundaries)
                    pss = []
                    ps = pacc.tile([128, obs], F32, name="ps_acc", tag="acc")
                    for ktp in range(nt // 2):
                        nc.tensor.matmul(
                            ps[:, :],
                            lhsT=strip[:, ktp * 256:(ktp + 1) * 256].rearrange(
                                "p (kt2 j) -> p kt2 j", kt2=2),
                            rhs=upair(ktp, src),
                            start=(ktp == 0),
                            stop=(ktp == nt // 2 - 1),
                            perf_mode=mybir.MatmulPerfMode.DoubleRow,
                        )
                    pss.append(ps)
                    if first_v:
                        # V = v0 + s0 @ c0   (V layout: b*o + oo, matches psum)
                        nc.vector.tensor_add(V[t][:, :], pss[0][:, :], uslot(t, 0))
                        for ps in pss[1:]:
                            nc.vector.tensor_add(V[t][:, :], V[t][:, :], ps[:, :])
                    elif final:
                        for ps in pss:
                            nc.vector.tensor_add(V[t][:, :], V[t][:, :], ps[:, :])
                        nc.sync.dma_start(
                            out=out[t * 128:(t + 1) * 128, :], in_=V[t][:, :]
                        )
                    else:
                        # psum = (256*s0)@(2u2/16) = 16*(2 s0 u2); slot1 = 16*u1
                        # -> plain add keeps c0 at 16x scale (fp8-safe)
                        d = uslot_w(t, dst_slot)
                        for ps in pss:
                            nc.vector.tensor_add(d, d, ps[:, :])

            hops = [
                (s0t, 2, 1, False, False, 1),    # c0 = u1 + s0 @ (2 u2)
                (s1t, 4, 3, False, False, 1),    # c1 = u3 + s1 @ (2 u4)
                (s0t, 1, None, True, False, 1),  # V = v0 + s0 @ c0
                (s1t, 3, None, False, True, 1),  # V += s1 @ c1 ; dma out
            ]
            # Final phase: hops 3+4 merged into one 32-matmul accumulation
            # per output tile (V = v0 + s0@c0 + s1@c1 with a single psum
            # group) — fewer adds, V written once, then streamed out.
            def final_merged():
                for t in range(nt):
                    strip0 = stream.tile([128, n], FP8, name="strip", tag="strip")
                    nc.sync.dma_start(out=strip0[:, :], in_=s0t[t])
                    strip1 = stream.tile([128, n], FP8, name="strip", tag="strip")
                    nc.sync.dma_start(out=strip1[:, :], in_=s1t[t])
                    ps = pacc.tile([128, obs], F32, name="ps_acc", tag="acc")
                    for g, (sb, sl) in enumerate([(strip0, 1), (strip1, 3)]):
                        for ktp in range(nt // 2):
                            nc.tensor.matmul(
                                ps[:, :],
                                lhsT=sb[:, ktp * 256:(ktp + 1) * 256].rearrange(
                                    "p (kt2 j) -> p kt2 j", kt2=2),
                                rhs=upair(ktp, sl),
                                start=(g == 0 and ktp == 0),
                                stop=(g == 1 and ktp == nt // 2 - 1),
                                perf_mode=mybir.MatmulPerfMode.DoubleRow,
                            )
                    nc.vector.scalar_tensor_tensor(
                        out=V[t][:, :], in0=ps[:, :], scalar=1.0 / (SCALE * 16.0),
                        op0=mybir.AluOpType.mult,
                        in1=V0[t][:, :], op1=mybir.AluOpType.add)
                    nc.sync.dma_start(
                        out=out[t * 128:(t + 1) * 128, :], in_=V[t][:, :]
                    )

            # repeat>1 re-runs the whole idempotent pipeline (each round
            # rebuilds U from x0 and recreates V) — used only to measure
            # per-round device time via wall-clock differencing.
            for _rep in range(repeat):
                for t in range(nt):
                    phase1(t)
                if n_hops >= 4:
                    for hargs in hops[:2]:
                        hop(*hargs)
                    final_merged()
                else:
                    for hargs in hops[:n_hops]:
                        hop(*hargs)
    return _legalize_waits(nc) if legalize else nc


_NC_CACHE = {}


def _get_nc():
    if "nc" not in _NC_CACHE:
        _NC_CACHE["nc"] = build_bass()
    return _NC_CACHE["nc"]


def make_inputs(support0, support1, inputs, state, weight):
    """Host-side layout prep -> per-core in_maps (shared replicated arrays)."""
    xs = np.concatenate(
        [
            np.asarray(inputs, np.float32).reshape(B, N, F // 2),
            np.asarray(state, np.float32).reshape(B, N, F // 2),
        ],
        axis=2,
    )  # [B, N, F]

    w = np.asarray(weight, np.float32).reshape(F, M5, O)
    wv0 = w[:, 0] - w[:, 2] - w[:, 4]
    wcat = np.concatenate(
        [wv0, 16.0 * w[:, 1], 2.0 * w[:, 2] / 16.0,
         16.0 * w[:, 3], 2.0 * w[:, 4] / 16.0], axis=1
    ).astype(NPBF16)  # [128, 320]; hop slots scaled so fp8 adds stay in-range

    def strip_img(s):
        # fp8 DoubleRow pair layout: [t, p, ktp*256 + kt2*128 + j]
        #   = fp8(SCALE * s[t*128+j, (ktp*2+kt2)*128 + p])
        r = (SCALE * np.asarray(s, np.float32)).astype(NPFP8)
        r = r.reshape(NT, 128, NT, 128).transpose(0, 3, 2, 1)  # [t, p, kt, j]
        return np.ascontiguousarray(r.reshape(NT, 128, N))

    s0i, s1i = strip_img(support0), strip_img(support1)

    in_maps = []
    for c in range(NCORES):
        shard = xs[c * BS:(c + 1) * BS]                # [8b, N, F]
        # t-major SBUF image: x0t[f, t*BS*128 + b*128 + j] = shard[b, t*128+j, f]
        x0t = np.ascontiguousarray(
            shard.reshape(BS, NT, 128, F).transpose(3, 1, 0, 2).reshape(F, BS * N)
        ).astype(NPBF16)
        in_maps.append({"x0t": x0t, "wcat": wcat, "s0t": s0i, "s1t": s1i})
    return in_maps


def postprocess(results, biases):
    full = np.empty((B, N, O), np.float32)
    for c, r in enumerate(results):
        full[c * BS:(c + 1) * BS] = (
            r["out"].reshape(N, BS, O).transpose(1, 0, 2)
        )
    full += np.asarray(biases, np.float32)[None, None, :]
    return full.reshape(B, N * O)


def kernel(support0, support1, inputs, state, weight, biases, output_size=None,
           **run_kwargs):
    nc = _get_nc()
    in_maps = make_inputs(support0, support1, inputs, state, weight)
    res = run_bass_kernel_spmd(nc, in_maps, core_ids=list(range(NCORES)),
                               **run_kwargs)
    out = postprocess(res.results, biases)
    if run_kwargs.get("trace"):
        return out, res
    return out



# revision 3
# speedup vs baseline: 942.0739x; 942.0739x over previous
"""DiffusionGraphConv Trainium2 kernel (8-core SPMD, data-parallel over batch).

Math refactoring (halves the big-matmul FLOPs vs the reference order):
  reference: out[b,n,o] = sum_{f,m} mats_m[n,f,b] * W[f*5+m, o]
  with mats = [x0, s0 x0, 2 s0^2 x0 - x0, s1 x0, 2 s1^2 x0 - x0].
  Projection (width F=128 -> O=64) commutes with the node-space diffusion, so
  with u_m = x0 @ W_m:
    out = (u0 - u2 - u4) + s0 (u1 + 2 s0 u2) + s1 (u3 + 2 s1 u4)
        = v0 + s0 @ c0 + s1 @ c1,  c0 = u1 + s0 @ (2 u2), c1 = u3 + s1 @ (2 u4)

Execution structure (v2 — copy/DMA-lean):
  * u1 / u3 / v0 are never materialized: they are folded into the diffusion
    PSUM groups as 8 extra 64-col bf16 matmuls (lhsT = x0 tile, rhs = a
    pre-scaled W slice) appended after the fp8 DoubleRow strip matmuls.
    Phase 1 therefore only projects/stores slots 2u2, 2u4 (fp8, 16x-scaled),
    and each hop ends in ONE psum->SBUF copy instead of add+copy.
  * Both support strip sets stay SBUF-resident (loaded once, 64KB/partition),
    so strip HBM traffic is 8MB/core instead of 16MB and the final phase has
    no DMA dependence at all.
  * PSUM->SBUF copies rotate across DVE / Activation / Pool(gpsimd).
  * Output is written bf16 (host accumulates in f32 and adds biases).

Per-core work: 512 fp8-DoubleRow matmuls [128,2x128]@[128,2,512] + 640 small
bf16 matmuls; PE busy ~72us is the modeled bottleneck.

Scales (fp8-safety): strips = fp8(256*s); slot2/4 = fp8(2*u{2,4}/16);
hop psum = 16*c{0,1} stored fp8; final psum = 256*16*(s@c) + 4096*v0
= 4096*out, drained with a 1/4096 scaled copy.

Env quirks handled here: walrus accepts <=1 sync-wait per instruction
(_legalize_waits hoists extras onto EventSemaphore carriers; simulators need
legalize=False); repeat=N re-runs the idempotent pipeline for wall-clock
differencing since this axon terminal has no NTFF profiling.
"""

import sys

if "/opt/trn_rl_repo" not in sys.path:
    sys.path.insert(0, "/opt/trn_rl_repo")

import numpy as np
import ml_dtypes

import concourse.bass as bass
import concourse.mybir as mybir
from concourse.tile import TileContext
from concourse.bass_utils import run_bass_kernel_spmd

BF16 = mybir.dt.bfloat16
FP8 = mybir.dt.float8e4
NPFP8 = ml_dtypes.float8_e4m3
SCALE = 256.0
FOLD = 16.0                 # u1/u3 fold scale (matches strip*slot scale)
VSCALE = SCALE * FOLD       # 4096: final psum = VSCALE * out
F32 = mybir.dt.float32
NPBF16 = ml_dtypes.bfloat16

N = 2048          # graph nodes
F = 128           # input_size (64 input + 64 hidden)
B = 64            # global batch
NCORES = 8
BS = B // NCORES  # 8 batches per core
O = 64            # output features
NT = N // 128     # 16 node tiles
M5 = 5            # diffusion matrices


def _legalize_waits(nc, max_waits=1):
    """Walrus in this env encodes at most one sync-wait per instruction.

    Tile's sem assignment can emit 2-3 waits on one instruction; hoist the
    excess onto standalone EventSemaphore carriers (same engine, inserted
    just before), which the sequencer executes in order — semantics are
    identical, encoding is legal."""
    f = nc.m.functions[0]
    for blk in f.blocks:
        new_insts = []
        changed = False
        for inst in blk.instructions:
            si = inst.sync_info
            waits = list(si.on_wait) if si is not None else []
            if len(waits) > max_waits:
                for i, w in enumerate(waits[:-max_waits]):
                    ev = mybir.InstEventSemaphore(
                        name=f"{inst.name}-wsplit{i}",
                        engine=inst.engine,
                        ins=[],
                        outs=[],
                        sync_info=mybir.SyncInfo(on_wait=[w], on_update=[]),
                    )
                    new_insts.append(ev)
                inst.sync_info = mybir.SyncInfo(
                    on_wait=waits[-max_waits:], on_update=list(si.on_update)
                )
                changed = True
            new_insts.append(inst)
        if changed:
            blk.instructions = new_insts
    return nc


def build_bass(n=N, bs=BS, o=O, legalize=True, repeat=1):
    """Build the per-core SPMD Bass program."""
    nt = n // 128
    nc = bass.Bass()
    x0t = nc.dram_tensor("x0t", [F, bs * n], BF16, kind="ExternalInput")
    # wcat slices (64 cols each): [2W2/16 | 2W4/16 | 16W1 | 16W3 | 4096*Wv0]
    wcat = nc.dram_tensor("wcat", [F, M5 * o], BF16, kind="ExternalInput")
    s0t = nc.dram_tensor("s0t", [nt, 128, n], FP8, kind="ExternalInput")
    s1t = nc.dram_tensor("s1t", [nt, 128, n], FP8, kind="ExternalInput")
    out = nc.dram_tensor("out", [n, bs * o], BF16, kind="ExternalOutput")

    obs = bs * o        # 512: width of diffusion operands
    with TileContext(nc) as tc:
        with (
            tc.tile_pool(name="persist", bufs=1) as persist,
            tc.tile_pool(name="outp", bufs=4) as outp,
            tc.tile_pool(name="pproj", bufs=4, space="PSUM") as pproj,
            tc.tile_pool(name="pacc", bufs=4, space="PSUM") as pacc,
        ):
            w_sb = persist.tile([F, M5 * o], BF16, name="w_sb")
            nc.sync.dma_start(out=w_sb[:, :], in_=wcat[:, :])
            # x0t is t-major on host: free index = t*bs*128 + b*128 + j, so
            # each node-tile's stationary slices arrive in one chunk DMA.
            x0_sb = persist.tile([F, bs * n], BF16, name="x0_sb")
            for t in range(nt):
                nc.sync.dma_start(
                    out=x0_sb[:, t * bs * 128:(t + 1) * bs * 128],
                    in_=x0t[:, t * bs * 128:(t + 1) * bs * 128],
                )
            # Resident strip sets: loaded once, reused by hop and final phases.
            S0 = [persist.tile([128, n], FP8, name=f"s0_{t}", tag=f"s0_{t}")
                  for t in range(nt)]
            S1 = [persist.tile([128, n], FP8, name=f"s1_{t}", tag=f"s1_{t}")
                  for t in range(nt)]
            for t in range(nt):
                nc.sync.dma_start(out=S0[t][:, :], in_=s0t[t])
            for t in range(nt):
                nc.sync.dma_start(out=S1[t][:, :], in_=s1t[t])
            # U[tp]: [128, 4*2*obs] fp8, slots (mi-1) in {0:c0, 1:2u2/16,
            # 2:c1, 3:2u4/16}, each slot = [kt2, b*o].
            U = [
                persist.tile([128, 4 * 2 * obs], FP8, name=f"u{tp}", tag=f"u{tp}")
                for tp in range(nt // 2)
            ]

            def upair(tp, mi):
                """[128, 2, obs] DoubleRow moving view: k-tile pair of slot mi."""
                return U[tp].rearrange("p (mi4 kt2 c) -> p mi4 kt2 c", mi4=4, kt2=2)[
                    :, mi - 1, :, :
                ]

            def uslot_w(t, mi):
                """[128, obs] contiguous write view of slot mi for node-tile t."""
                base = (mi - 1) * 2 * obs + (t % 2) * obs
                return U[t // 2][:, base:base + obs]

            def xblk(t, b):
                """Stationary x0 slice for (node-tile t, batch b): [128f, 128n]."""
                return x0_sb[:, (t * bs + b) * 128:(t * bs + b + 1) * 128]

            # PSUM->SBUF drains rotate across DVE and Activation (gpsimd/Pool
            # cannot read PSUM).
            eng_state = [0]

            def drain(out_ap, in_ap, scale=None):
                e = eng_state[0] % 2
                eng_state[0] += 1
                if scale is None:
                    if e == 0:
                        nc.vector.tensor_copy(out=out_ap, in_=in_ap)
                    else:
                        nc.scalar.copy(out=out_ap, in_=in_ap)
                else:
                    if e == 0:
                        nc.vector.tensor_scalar_mul(out_ap, in_ap, scale)
                    else:
                        nc.scalar.activation(
                            out_ap, in_ap,
                            mybir.ActivationFunctionType.Copy, scale=scale)

            # ---- Phase 1: project slots 2u2/16 and 2u4/16 (b-packed banks,
            # one contiguous 512-col fp8 copy per slot per node-tile).
            def phase1(t):
                ps2 = pproj.tile([128, obs], F32, name="ps2", tag="proj")
                ps4 = pproj.tile([128, obs], F32, name="ps4", tag="proj")
                for b in range(bs):
                    nc.tensor.matmul(
                        ps2[:, b * o:(b + 1) * o], lhsT=xblk(t, b),
                        rhs=w_sb[:, 0:o], start=True, stop=True)
                    nc.tensor.matmul(
                        ps4[:, b * o:(b + 1) * o], lhsT=xblk(t, b),
                        rhs=w_sb[:, o:2 * o], start=True, stop=True)
                drain(uslot_w(t, 2), ps2[:, :])
                drain(uslot_w(t, 4), ps4[:, :])

            # ---- Hops: c_dst = u_w + s @ (2 u_src)  [all at 16x scale]
            #   8 fp8 DoubleRow strip matmuls + 8 bf16 64-col fold matmuls
            #   accumulate in one psum group; single fp8 copy out.
            def hop(S, src, wi, dst):
                for t in range(nt):
                    ps = pacc.tile([128, obs], F32, name="ps_acc", tag="acc")
                    for ktp in range(nt // 2):
                        nc.tensor.matmul(
                            ps[:, :],
                            lhsT=S[t][:, ktp * 256:(ktp + 1) * 256].rearrange(
                                "p (kt2 j) -> p kt2 j", kt2=2),
                            rhs=upair(ktp, src),
                            start=(ktp == 0),
                            stop=False,
                            perf_mode=mybir.MatmulPerfMode.DoubleRow,
                        )
                    for b in range(bs):
                        nc.tensor.matmul(
                            ps[:, b * o:(b + 1) * o], lhsT=xblk(t, b),
                            rhs=w_sb[:, wi * o:(wi + 1) * o],
                            start=False, stop=(b == bs - 1))
                    drain(uslot_w(t, dst), ps[:, :])

            # ---- Final: psum = 4096*(s0@c0 + s1@c1 + v0); scaled bf16 drain.
            def final():
                for t in range(nt):
                    ps = pacc.tile([128, obs], F32, name="ps_acc", tag="acc")
                    for g, (S, sl) in enumerate([(S0, 1), (S1, 3)]):
                        for ktp in range(nt // 2):
                            nc.tensor.matmul(
                                ps[:, :],
                                lhsT=S[t][:, ktp * 256:(ktp + 1) * 256].rearrange(
                                    "p (kt2 j) -> p kt2 j", kt2=2),
                                rhs=upair(ktp, sl),
                                start=(g == 0 and ktp == 0),
                                stop=False,
                                perf_mode=mybir.MatmulPerfMode.DoubleRow,
                            )
                    for b in range(bs):
                        nc.tensor.matmul(
                            ps[:, b * o:(b + 1) * o], lhsT=xblk(t, b),
                            rhs=w_sb[:, 4 * o:M5 * o],
                            start=False, stop=(b == bs - 1))
                    ot = outp.tile([128, obs], BF16, name="ot", tag="ot")
                    drain(ot[:, :], ps[:, :], scale=1.0 / VSCALE)
                    nc.sync.dma_start(
                        out=out[t * 128:(t + 1) * 128, :], in_=ot[:, :])

            # repeat>1 re-runs the whole idempotent pipeline (each round
            # rebuilds U from x0) — used only to measure per-round device
            # time via wall-clock differencing.
            for _rep in range(repeat):
                for t in range(nt):
                    phase1(t)
                hop(S0, 2, 2, 1)   # c0 = u1 + s0 @ (2 u2)
                hop(S1, 4, 3, 3)   # c1 = u3 + s1 @ (2 u4)
                final()
    return _legalize_waits(nc) if legalize else nc


_NC_CACHE = {}


def _get_nc():
    if "nc" not in _NC_CACHE:
        _NC_CACHE["nc"] = build_bass()
    return _NC_CACHE["nc"]


def make_inputs(support0, support1, inputs, state, weight):
    """Host-side layout prep -> per-core in_maps (shared replicated arrays)."""
    xs = np.concatenate(
        [
            np.asarray(inputs, np.float32).reshape(B, N, F // 2),
            np.asarray(state, np.float32).reshape(B, N, F // 2),
        ],
        axis=2,
    )  # [B, N, F]

    w = np.asarray(weight, np.float32).reshape(F, M5, O)
    wv0 = w[:, 0] - w[:, 2] - w[:, 4]
    wcat = np.concatenate(
        [2.0 * w[:, 2] / FOLD, 2.0 * w[:, 4] / FOLD,
         FOLD * w[:, 1], FOLD * w[:, 3], VSCALE * wv0], axis=1
    ).astype(NPBF16)  # [128, 320]; slot scales keep fp8 operands in-range

    def strip_img(s):
        # fp8 DoubleRow pair layout: [t, p, ktp*256 + kt2*128 + j]
        #   = fp8(SCALE * s[t*128+j, (ktp*2+kt2)*128 + p])
        r = (SCALE * np.asarray(s, np.float32)).astype(NPFP8)
        r = r.reshape(NT, 128, NT, 128).transpose(0, 3, 2, 1)  # [t, p, kt, j]
        return np.ascontiguousarray(r.reshape(NT, 128, N))

    s0i, s1i = strip_img(support0), strip_img(support1)

    in_maps = []
    for c in range(NCORES):
        shard = xs[c * BS:(c + 1) * BS]                # [8b, N, F]
        # t-major SBUF image: x0t[f, t*BS*128 + b*128 + j] = shard[b, t*128+j, f]
        x0t = np.ascontiguousarray(
            shard.reshape(BS, NT, 128, F).transpose(3, 1, 0, 2).reshape(F, BS * N)
        ).astype(NPBF16)
        in_maps.append({"x0t": x0t, "wcat": wcat, "s0t": s0i, "s1t": s1i})
    return in_maps


def postprocess(results, biases):
    full = np.empty((B, N, O), np.float32)
    for c, r in enumerate(results):
        full[c * BS:(c + 1) * BS] = (
            r["out"].astype(np.float32).reshape(N, BS, O).transpose(1, 0, 2)
        )
    full += np.asarray(biases, np.float32)[None, None, :]
    return full.reshape(B, N * O)


def kernel(support0, support1, inputs, state, weight, biases, output_size=None,
           **run_kwargs):
    nc = _get_nc()
    in_maps = make_inputs(support0, support1, inputs, state, weight)
    res = run_bass_kernel_spmd(nc, in_maps, core_ids=list(range(NCORES)),
                               **run_kwargs)
    out = postprocess(res.results, biases)
    if run_kwargs.get("trace"):
        return out, res
    return out
